# revision 14
# baseline (speedup 1.0000x reference)
"""Trainium2 Bass kernel v3: baseline per-block pipeline + surgical changes.

vs v1 (139.5us):
  - bn_aggr per subtile (18.7us DVE) replaced by per-block batched even/odd
    merge on [128, 8] tensors:
      S = m_e+m_o (=2mu), D = m_e-m_o, M2tot = (M_e+M_o) + 128 D^2
      sig = sqrt(M2tot/512 + eps) [ACT], rstd = 1/sig, nmr = -0.5 S rstd
  - output DRAM layout [128, 128, 512] -> 8KB contiguous per-partition DMA
    lines; host un-shuffles
  - norm split 4 ACT / 4 DVE per block (as v1)
"""

import numpy as np
import ml_dtypes

import concourse.bass as bass
import concourse.tile as tile
from concourse import bacc, mybir
from concourse.bass_utils import run_bass_kernel_spmd



def _ensure_ntff_hook():
    """Inject antenv.axon_hooks (missing in this image) so that
    run_bass_kernel_spmd(trace=True) works instead of raising ImportError."""
    try:
        from antenv.axon_hooks import get_axon_ntff_profile_hook  # noqa: F401
        return
    except ImportError:
        pass
    try:
        import contextlib
        import ctypes
        import sys
        import types

        lib = ctypes.CDLL("/opt/axon/libaxon_pjrt.so")
        if not hasattr(lib, "axon_start_nrt_profile"):
            return
        lib.axon_start_nrt_profile.argtypes = [
            ctypes.POINTER(ctypes.c_int64), ctypes.c_size_t]
        lib.axon_start_nrt_profile.restype = ctypes.c_int64
        lib.axon_stop_nrt_profile.argtypes = [ctypes.c_char_p]
        lib.axon_stop_nrt_profile.restype = ctypes.c_int64

        @contextlib.contextmanager
        def _hook(output_dir, device_ids):
            import jax
            jax.devices()
            if device_ids:
                ids = (ctypes.c_int64 * len(device_ids))(*device_ids)
                rc = lib.axon_start_nrt_profile(ids, len(device_ids))
            else:
                rc = lib.axon_start_nrt_profile(None, 0)
            if rc != 0:
                raise RuntimeError(f"axon_start_nrt_profile rc={rc}")
            try:
                yield
            finally:
                lib.axon_stop_nrt_profile(str(output_dir).encode())

        import antenv
        mod = types.ModuleType("antenv.axon_hooks")
        mod.get_axon_ntff_profile_hook = lambda: _hook
        mod.set_axon_ntff_profile_hook = lambda h: None
        sys.modules["antenv.axon_hooks"] = mod
        antenv.axon_hooks = mod
    except Exception:
        pass


_ensure_ntff_hook()

R, F, IN, OUT_TOT = 4096, 32, 256, 512
N_CORES = 8
TOKENS = R * F
TPC = TOKENS // N_CORES          # 16384
KC = IN // 128                   # 2
BLK = 1024
NBLK = TPC // BLK                # 16
SUB = BLK // 128                 # 8
GRP = 4
EPS = 1e-5
NEG_SLOPE = 0.01
BF16 = mybir.dt.bfloat16
F32 = mybir.dt.float32

_compiled = {}


def _build_nc():
    nc = bacc.Bacc(None)
    xT = nc.declare_dram_parameter("xT", [KC, 128, TPC], BF16, isOutput=False)
    w = nc.declare_dram_parameter("w", [KC, 128, OUT_TOT], BF16, isOutput=False)
    y = nc.declare_dram_parameter("y", [128, NBLK * SUB, OUT_TOT], BF16,
                                  isOutput=True)

    with tile.TileContext(nc) as tc:
        with (
            tc.tile_pool(name="singles", bufs=1) as singles,
            tc.tile_pool(name="xpool", bufs=4) as xpool,
            tc.tile_pool(name="vpool", bufs=6) as vpool,
            tc.tile_pool(name="opool", bufs=6) as opool,
            tc.tile_pool(name="stats", bufs=8) as stats_pool,
            tc.tile_pool(name="psum", bufs=2, space="PSUM") as psum,
        ):
            w_sb = singles.tile([128, KC, OUT_TOT], BF16)
            nc.sync.dma_start(out=w_sb, in_=w[:].rearrange("c k n -> k c n"))
            eps_sb = singles.tile([128, 1], F32)
            nc.vector.memset(eps_sb, EPS)

            # schedule: first/last blocks split in half to shrink ramp/tail
            sched = [(0, 4), (4, 4)] + [(8 * b, 8) for b in range(1, NBLK - 1)] \
                + [(8 * (NBLK - 1), 4), (8 * (NBLK - 1) + 4, 4)]
            x_pair = [None, 0]
            for se_i, (s0, nsub) in enumerate(sched):
                tok0 = s0 * 128
                if se_i % 2 == 0:
                    nsub2 = nsub + (sched[se_i + 1][1] if se_i + 1 < len(sched)
                                    else 0)
                    xp = xpool.tile([128, KC, nsub2 * 128], BF16, name="x_sb")
                    nc.sync.dma_start(
                        out=xp,
                        in_=xT[:, :, tok0:tok0 + nsub2 * 128].rearrange(
                            "c k t -> k c t"),
                    )
                    x_pair = [xp, 0]
                x_sb = x_pair[0][:, :, x_pair[1]:x_pair[1] + nsub * 128]
                x_pair[1] += nsub * 128
                v_sb = vpool.tile([128, nsub, OUT_TOT], BF16, name="v_sb")
                o_sb = opool.tile([128, nsub, OUT_TOT], BF16, name="o_sb")
                st = stats_pool.tile([128, nsub, 6], F32, name="st")

                for g in range(nsub // GRP):
                    ps = psum.tile([128, GRP, OUT_TOT], F32, name="ps")
                    for j in range(GRP):
                        i = g * GRP + j
                        nc.tensor.matmul(
                            ps[:, j, :], lhsT=x_sb[:, 0, bass.ts(i, 128)],
                            rhs=w_sb[:, 0, :], start=True, stop=False,
                        )
                        nc.tensor.matmul(
                            ps[:, j, :], lhsT=x_sb[:, 1, bass.ts(i, 128)],
                            rhs=w_sb[:, 1, :], start=False, stop=True,
                        )
                    nc.scalar.activation(
                        v_sb[:, g * GRP:(g + 1) * GRP, :], ps,
                        mybir.ActivationFunctionType.Prelu, alpha=NEG_SLOPE,
                    )
                    for j in range(GRP):
                        i = g * GRP + j
                        nc.vector.bn_stats(st[:, i, :], v_sb[:, i, :])

                # batched even/odd merge on [128, nsub]
                S = stats_pool.tile([128, nsub], F32, name="S")
                Dd = stats_pool.tile([128, nsub], F32, name="Dd")
                M2 = stats_pool.tile([128, nsub], F32, name="M2")
                sig = stats_pool.tile([128, nsub], F32, name="sig")
                rstd = stats_pool.tile([128, nsub], F32, name="rstd")
                nmr = stats_pool.tile([128, nsub], F32, name="nmr")
                nc.vector.tensor_tensor(
                    S, st[:, :, 1], st[:, :, 4], mybir.AluOpType.add)
                nc.vector.tensor_tensor(
                    Dd, st[:, :, 1], st[:, :, 4], mybir.AluOpType.subtract)
                nc.vector.tensor_tensor(
                    M2, st[:, :, 2], st[:, :, 5], mybir.AluOpType.add)
                nc.vector.scalar_tensor_tensor(
                    out=Dd, in0=Dd, scalar=128.0, in1=Dd,
                    op0=mybir.AluOpType.mult, op1=mybir.AluOpType.mult)
                nc.vector.tensor_tensor(M2, M2, Dd, mybir.AluOpType.add)
                nc.scalar.activation(
                    sig, M2, mybir.ActivationFunctionType.Sqrt,
                    bias=eps_sb, scale=1.0 / OUT_TOT)
                nc.vector.reciprocal(rstd, sig)
                nc.vector.scalar_tensor_tensor(
                    out=nmr, in0=S, scalar=-0.5, in1=rstd,
                    op0=mybir.AluOpType.mult, op1=mybir.AluOpType.mult)

                for i in range(nsub):
                    if i % 2 == 0:
                        nc.vector.tensor_scalar(
                            out=o_sb[:, i, :], in0=v_sb[:, i, :],
                            scalar1=rstd[:, i:i + 1], scalar2=nmr[:, i:i + 1],
                            op0=mybir.AluOpType.mult,
                            op1=mybir.AluOpType.add,
                        )
                    else:
                        nc.scalar.activation(
                            o_sb[:, i, :], v_sb[:, i, :],
                            mybir.ActivationFunctionType.Identity,
                            bias=nmr[:, i:i + 1], scale=rstd[:, i:i + 1],
                        )
                nc.sync.dma_start(
                    out=y[:, s0:s0 + nsub, :], in_=o_sb)
    nc.finalize()
    return nc


def _get_nc():
    if "nc" not in _compiled:
        _compiled["nc"] = _build_nc()
    return _compiled["nc"]


def _in_maps(x, W_v, W_r):
    x = np.asarray(x, dtype=np.float32)
    W = (np.asarray(W_v, dtype=np.float32).reshape(IN, OUT_TOT)
         + np.asarray(W_r, dtype=np.float32))
    w_dev = np.ascontiguousarray(
        W.reshape(KC, 128, OUT_TOT).astype(ml_dtypes.bfloat16))

    xs = x.reshape(TOKENS, IN)
    in_maps = []
    for c in range(N_CORES):
        shard = xs[c * TPC:(c + 1) * TPC]
        xT = np.ascontiguousarray(shard.T.astype(ml_dtypes.bfloat16))
        in_maps.append({"xT": xT.reshape(KC, 128, TPC), "w": w_dev})
    return in_maps


def _gather(res):
    parts = []
    for c in range(N_CORES):
        yd = np.asarray(res.results[c]["y"])  # [128, 128, 512]
        full = yd.reshape(128, NBLK, SUB, OUT_TOT).transpose(1, 2, 0, 3)
        parts.append(full.reshape(TPC, OUT_TOT))
    out = np.concatenate(parts, axis=0)
    return out.reshape(R, F, OUT_TOT).astype(np.float32)


def kernel(x, W_q, W_k, W_v, W_r, ln_gamma, ln_beta):
    nc = _get_nc()
    in_maps = _in_maps(x, W_v, W_r)
    res = run_bass_kernel_spmd(nc, in_maps, list(range(N_CORES)))
    out = _gather(res)

    gamma = np.asarray(ln_gamma, dtype=np.float32)
    beta = np.asarray(ln_beta, dtype=np.float32)
    if not (np.all(gamma == 1.0) and np.all(beta == 0.0)):
        out = out * gamma + beta
    return out.astype(np.float32)


# revision 15
# speedup vs baseline: 1.0133x; 1.0133x over previous
"""Trainium2 Bass kernel v3: baseline per-block pipeline + surgical changes.

vs v1 (139.5us):
  - bn_aggr per subtile (18.7us DVE) replaced by per-block batched even/odd
    merge on [128, 8] tensors:
      S = m_e+m_o (=2mu), D = m_e-m_o, M2tot = (M_e+M_o) + 128 D^2
      sig = sqrt(M2tot/512 + eps) [ACT], rstd = 1/sig, nmr = -0.5 S rstd
  - output DRAM layout [128, 128, 512] -> 8KB contiguous per-partition DMA
    lines; host un-shuffles
  - norm split 4 ACT / 4 DVE per block (as v1)
"""

import numpy as np
import ml_dtypes

import concourse.bass as bass
import concourse.tile as tile
from concourse import bacc, mybir
from concourse.bass_utils import run_bass_kernel_spmd



def _ensure_ntff_hook():
    """Inject antenv.axon_hooks (missing in this image) so that
    run_bass_kernel_spmd(trace=True) works instead of raising ImportError."""
    try:
        from antenv.axon_hooks import get_axon_ntff_profile_hook  # noqa: F401
        return
    except ImportError:
        pass
    try:
        import contextlib
        import ctypes
        import sys
        import types

        lib = ctypes.CDLL("/opt/axon/libaxon_pjrt.so")
        if not hasattr(lib, "axon_start_nrt_profile"):
            return
        lib.axon_start_nrt_profile.argtypes = [
            ctypes.POINTER(ctypes.c_int64), ctypes.c_size_t]
        lib.axon_start_nrt_profile.restype = ctypes.c_int64
        lib.axon_stop_nrt_profile.argtypes = [ctypes.c_char_p]
        lib.axon_stop_nrt_profile.restype = ctypes.c_int64

        @contextlib.contextmanager
        def _hook(output_dir, device_ids):
            import jax
            jax.devices()
            if device_ids:
                ids = (ctypes.c_int64 * len(device_ids))(*device_ids)
                rc = lib.axon_start_nrt_profile(ids, len(device_ids))
            else:
                rc = lib.axon_start_nrt_profile(None, 0)
            if rc != 0:
                raise RuntimeError(f"axon_start_nrt_profile rc={rc}")
            try:
                yield
            finally:
                lib.axon_stop_nrt_profile(str(output_dir).encode())

        import antenv
        mod = types.ModuleType("antenv.axon_hooks")
        mod.get_axon_ntff_profile_hook = lambda: _hook
        mod.set_axon_ntff_profile_hook = lambda h: None
        sys.modules["antenv.axon_hooks"] = mod
        antenv.axon_hooks = mod
    except Exception:
        pass


_ensure_ntff_hook()

R, F, IN, OUT_TOT = 4096, 32, 256, 512
N_CORES = 8
TOKENS = R * F
TPC = TOKENS // N_CORES          # 16384
KC = IN // 128                   # 2
BLK = 1024
NBLK = TPC // BLK                # 16
SUB = BLK // 128                 # 8
GRP = 4
EPS = 1e-5
NEG_SLOPE = 0.01
BF16 = mybir.dt.bfloat16
F32 = mybir.dt.float32

_compiled = {}


def _build_nc():
    nc = bacc.Bacc(None)
    xT = nc.declare_dram_parameter("xT", [KC, 128, TPC], BF16, isOutput=False)
    w = nc.declare_dram_parameter("w", [KC, 128, OUT_TOT], BF16, isOutput=False)
    y = nc.declare_dram_parameter("y", [128, NBLK * SUB, OUT_TOT], BF16,
                                  isOutput=True)

    with tile.TileContext(nc) as tc:
        with (
            tc.tile_pool(name="singles", bufs=1) as singles,
            tc.tile_pool(name="xpool", bufs=4) as xpool,
            tc.tile_pool(name="vpool", bufs=6) as vpool,
            tc.tile_pool(name="opool", bufs=6) as opool,
            tc.tile_pool(name="stats", bufs=8) as stats_pool,
            tc.tile_pool(name="psum", bufs=2, space="PSUM") as psum,
        ):
            w_sb = singles.tile([128, KC, OUT_TOT], BF16)
            nc.sync.dma_start(out=w_sb, in_=w[:].rearrange("c k n -> k c n"))
            eps_sb = singles.tile([128, 1], F32)
            nc.vector.memset(eps_sb, EPS)

            # schedule: first/last blocks split in half to shrink ramp/tail
            sched = [(0, 4), (4, 4)] + [(8 * b, 8) for b in range(1, NBLK - 1)] \
                + [(8 * (NBLK - 1), 4), (8 * (NBLK - 1) + 4, 4)]
            for s0, nsub in sched:
                tok0 = s0 * 128
                x_sb = xpool.tile([128, KC, nsub * 128], BF16, name="x_sb")
                nc.sync.dma_start(
                    out=x_sb,
                    in_=xT[:, :, tok0:tok0 + nsub * 128].rearrange(
                        "c k t -> k c t"),
                )
                v_sb = vpool.tile([128, nsub, OUT_TOT], BF16, name="v_sb")
                o_sb = opool.tile([128, nsub, OUT_TOT], BF16, name="o_sb")
                st = stats_pool.tile([128, nsub, 6], F32, name="st")

                for g in range(nsub // GRP):
                    ps = psum.tile([128, GRP, OUT_TOT], F32, name="ps")
                    for j in range(GRP):
                        i = g * GRP + j
                        nc.tensor.matmul(
                            ps[:, j, :], lhsT=x_sb[:, 0, bass.ts(i, 128)],
                            rhs=w_sb[:, 0, :], start=True, stop=False,
                        )
                        nc.tensor.matmul(
                            ps[:, j, :], lhsT=x_sb[:, 1, bass.ts(i, 128)],
                            rhs=w_sb[:, 1, :], start=False, stop=True,
                        )
                    nc.scalar.activation(
                        v_sb[:, g * GRP:(g + 1) * GRP, :], ps,
                        mybir.ActivationFunctionType.Prelu, alpha=NEG_SLOPE,
                    )
                    for j in range(GRP):
                        i = g * GRP + j
                        nc.vector.bn_stats(st[:, i, :], v_sb[:, i, :])

                # batched even/odd merge on [128, nsub]
                S = stats_pool.tile([128, nsub], F32, name="S")
                Dd = stats_pool.tile([128, nsub], F32, name="Dd")
                M2 = stats_pool.tile([128, nsub], F32, name="M2")
                sig = stats_pool.tile([128, nsub], F32, name="sig")
                rstd = stats_pool.tile([128, nsub], F32, name="rstd")
                nmr = stats_pool.tile([128, nsub], F32, name="nmr")
                nc.vector.tensor_tensor(
                    S, st[:, :, 1], st[:, :, 4], mybir.AluOpType.add)
                nc.vector.tensor_tensor(
                    Dd, st[:, :, 1], st[:, :, 4], mybir.AluOpType.subtract)
                nc.vector.tensor_tensor(
                    M2, st[:, :, 2], st[:, :, 5], mybir.AluOpType.add)
                nc.vector.scalar_tensor_tensor(
                    out=Dd, in0=Dd, scalar=128.0, in1=Dd,
                    op0=mybir.AluOpType.mult, op1=mybir.AluOpType.mult)
                nc.vector.tensor_tensor(M2, M2, Dd, mybir.AluOpType.add)
                nc.scalar.activation(
                    sig, M2, mybir.ActivationFunctionType.Sqrt,
                    bias=eps_sb, scale=1.0 / OUT_TOT)
                nc.vector.reciprocal(rstd, sig)
                nc.vector.scalar_tensor_tensor(
                    out=nmr, in0=S, scalar=-0.5, in1=rstd,
                    op0=mybir.AluOpType.mult, op1=mybir.AluOpType.mult)

                for i in range(nsub):
                    if i % 2 == 0:
                        nc.vector.tensor_scalar(
                            out=o_sb[:, i, :], in0=v_sb[:, i, :],
                            scalar1=rstd[:, i:i + 1], scalar2=nmr[:, i:i + 1],
                            op0=mybir.AluOpType.mult,
                            op1=mybir.AluOpType.add,
                        )
                    else:
                        nc.scalar.activation(
                            o_sb[:, i, :], v_sb[:, i, :],
                            mybir.ActivationFunctionType.Identity,
                            bias=nmr[:, i:i + 1], scale=rstd[:, i:i + 1],
                        )
                nc.sync.dma_start(
                    out=y[:, s0:s0 + nsub, :], in_=o_sb)
    nc.finalize()
    return nc


def _get_nc():
    if "nc" not in _compiled:
        _compiled["nc"] = _build_nc()
    return _compiled["nc"]


def _in_maps(x, W_v, W_r):
    x = np.asarray(x, dtype=np.float32)
    W = (np.asarray(W_v, dtype=np.float32).reshape(IN, OUT_TOT)
         + np.asarray(W_r, dtype=np.float32))
    w_dev = np.ascontiguousarray(
        W.reshape(KC, 128, OUT_TOT).astype(ml_dtypes.bfloat16))

    xs = x.reshape(TOKENS, IN)
    in_maps = []
    for c in range(N_CORES):
        shard = xs[c * TPC:(c + 1) * TPC]
        xT = np.ascontiguousarray(shard.T.astype(ml_dtypes.bfloat16))
        in_maps.append({"xT": xT.reshape(KC, 128, TPC), "w": w_dev})
    return in_maps


def _gather(res):
    parts = []
    for c in range(N_CORES):
        yd = np.asarray(res.results[c]["y"])  # [128, 128, 512]
        full = yd.reshape(128, NBLK, SUB, OUT_TOT).transpose(1, 2, 0, 3)
        parts.append(full.reshape(TPC, OUT_TOT))
    out = np.concatenate(parts, axis=0)
    return out.reshape(R, F, OUT_TOT).astype(np.float32)


def kernel(x, W_q, W_k, W_v, W_r, ln_gamma, ln_beta):
    nc = _get_nc()
    in_maps = _in_maps(x, W_v, W_r)
    res = run_bass_kernel_spmd(nc, in_maps, list(range(N_CORES)))
    out = _gather(res)

    gamma = np.asarray(ln_gamma, dtype=np.float32)
    beta = np.asarray(ln_beta, dtype=np.float32)
    if not (np.all(gamma == 1.0) and np.all(beta == 0.0)):
        out = out * gamma + beta
    return out.astype(np.float32)
